# revision 1
# baseline (speedup 1.0000x reference)
"""Trainium2 Bass kernel for batched contrastive loss (InfoNCE over CxC sims).

Math (matches the jax reference):
    v_hat = v / ||v||,  t_hat = t / ||t||          (L2 over D, eps=1e-12)
    L[b,c,k] = (v_hat[b,c] . t_hat[b,k]) / 0.5     (logits)
    loss = mean_{b,c} [ logsumexp_k L[b,c,k] - L[b,c,c] ]

Strategy (8 NeuronCores, data-parallel over B=64 -> 8 batches/core):
  - SWDGE DMA loads PAIRS of batches (f32 -> bf16 cast in the DMA datapath)
    into SBUF [128p=c%128, 2 batch, 4 c-chunk, 256 d].
  - Norms: one fused mul + one fused multi-chunk reduce per tensor per pair.
  - rsqrt without the banned Rsqrt op and within ONE ACT table set
    (natural_log_exp_and_others, enforced via the get_activation_tables
    patch below):  sv = exp(-0.5*ln(nv2/4)) = 2/||v|| (1/temp folded),
    tsc = exp(-0.5*ln(nt2)) = 1/||t||.
  - T_hat = T * tsc via ONE tensor_tensor with a stride-0-broadcast AP.
  - Gram operands need D on partitions: PE identity-transpose of 128x128
    blocks into one PSUM bank per (tensor, batch), single [128,1024] copy out.
  - Gram: 4 c-chunks x 2 d-halves of bf16 matmuls -> PSUM [128,512] f32;
    exp + row-sum fused in one ACT op per chunk (per-partition scale AP = sv).
  - ln(rowsums) and the subtraction are hoisted out of the loop (one op each).
  - Each core returns per-(c,chunk,batch) loss terms [128,32]; host sums.
"""

import math
from contextlib import ExitStack

import numpy as np

import concourse.bacc as bacc
import concourse.bass as bass
import concourse.tile as tile
from concourse import mybir
from concourse.bass_utils import run_bass_kernel_spmd
from concourse.masks import make_identity

N_CORES = 8
B_PER_CORE = 8
PB = 2  # batches per DMA pair
NPAIR = B_PER_CORE // PB
C = 512
D = 256
P = 128
NCHUNK = C // P  # 4 c-chunks per batch
NDHALF = D // P  # 2 d-halves

F32 = mybir.dt.float32
BF16 = mybir.dt.bfloat16

# ---------------------------------------------------------------------------
# Keep ACT on a single table set: exp & ln both live in
# "natural_log_exp_and_others"; by removing them from every other set, the
# insert_act_table_loads fixpoint must pick that one set for both, so the
# kernel pays ONE table load instead of thrashing exp_and_others <->
# natural_log (~1.3us per reload, 33 reloads observed).  Set indices are
# preserved (membership edited, nothing reordered).
_orig_get_tables = bacc.get_activation_tables


def _patched_get_tables(arch):
    tables = dict(_orig_get_tables(arch))
    keep = "natural_log_exp_and_others"
    strip = {mybir.ActivationFunctionType.Exp, mybir.ActivationFunctionType.Ln}
    if keep in tables:
        for name in tables:
            if name != keep:
                tables[name] = set(tables[name]) - strip
    return tables


bacc.get_activation_tables = _patched_get_tables


def _bcast_cols(tile_ap, col0, ncols_outer, ncols_inner, bcast_count):
    """AP reading tile[:, col0 + o*ncols_inner + i] broadcast bcast_count
    times along a new innermost (stride-0) dim."""
    base = tile_ap[:, col0 : col0 + ncols_outer * ncols_inner]
    part_dim = base.ap[0]
    elem_step = base.ap[-1][0]
    return bass.AP(
        tensor=base.tensor,
        offset=base.offset,
        ap=[
            part_dim,
            [elem_step * ncols_inner, ncols_outer],
            [elem_step, ncols_inner],
            [0, bcast_count],
        ],
    )


def _emit(ctx: ExitStack, tc: tile.TileContext, loss_ap, v_ap, t_ap):
    nc = tc.nc
    # bf16 reduce outputs: the DVE accumulates in fp32 internally and only
    # rounds the [128,8] result; keeping every operand 2-byte makes the
    # reduce eligible for the 2x DVE perf mode. Error on ||.||^2 is ~0.4%,
    # ~4e-3 absolute on logits -- far inside the tolerance.
    ctx.enter_context(nc.allow_low_precision("bf16 norm/pos reduces, 2x DVE"))

    singles = ctx.enter_context(tc.tile_pool(name="singles", bufs=1))
    inputs = ctx.enter_context(tc.tile_pool(name="inputs", bufs=3))
    normed = ctx.enter_context(tc.tile_pool(name="normed", bufs=2))
    trans = ctx.enter_context(tc.tile_pool(name="trans", bufs=3))
    scratch = ctx.enter_context(tc.tile_pool(name="scratch", bufs=3))
    stats = ctx.enter_context(tc.tile_pool(name="stats", bufs=2))
    tp_pool = ctx.enter_context(tc.tile_pool(name="tp", bufs=4, space="PSUM"))
    gp_pool = ctx.enter_context(tc.tile_pool(name="gp", bufs=4, space="PSUM"))

    identity = singles.tile([P, P], BF16)
    make_identity(nc, identity)

    rs_all = singles.tile([P, NCHUNK * B_PER_CORE], F32)
    plog_all = singles.tile([P, NCHUNK * B_PER_CORE], F32)
    loss_cols = singles.tile([P, NCHUNK * B_PER_CORE], F32)

    for pair in range(NPAIR):
        b0 = pair * PB
        # ---- load a pair of batches (cast f32 -> bf16 in the DMA) ----
        V = inputs.tile([P, PB, NCHUNK, D], BF16, tag="V")
        T = inputs.tile([P, PB, NCHUNK, D], BF16, tag="T")
        nc.gpsimd.dma_start(
            out=V[:], in_=v_ap[b0 : b0 + PB].rearrange("b (n p) d -> p b n d", p=P)
        )
        nc.gpsimd.dma_start(
            out=T[:], in_=t_ap[b0 : b0 + PB].rearrange("b (n p) d -> p b n d", p=P)
        )

        # ---- transpose raw V early (independent of norms) ----
        Vt = trans.tile([P, PB, NDHALF, C], BF16, tag="Vt")
        for pb in range(PB):
            tpv = tp_pool.tile([P, NDHALF, C], BF16, tag="tp")
            for e in range(NDHALF):
                for j in range(NCHUNK):
                    nc.tensor.transpose(
                        tpv[:, e, j * P : (j + 1) * P],
                        V[:, pb, j, e * P : (e + 1) * P],
                        identity,
                    )
            nc.any.tensor_copy(out=Vt[:, pb], in_=tpv[:])

        # ---- fused norms: nvt2[:, 0:8]=|v|^2 (pb,j), [:, 8:16]=|t|^2 ----
        nvt2 = stats.tile([P, 2 * PB * NCHUNK], BF16, tag="nvt2")
        sqv = scratch.tile([P, PB, NCHUNK, D], BF16, tag="sq")
        nc.vector.tensor_mul(sqv[:], V[:], V[:])
        nc.vector.reduce_sum(
            nvt2[:, 0 : PB * NCHUNK].rearrange("p (b n) -> p b n", b=PB),
            sqv[:],
            axis=mybir.AxisListType.X,
        )
        sqt = scratch.tile([P, PB, NCHUNK, D], BF16, tag="sq")
        nc.vector.tensor_mul(sqt[:], T[:], T[:])
        nc.vector.reduce_sum(
            nvt2[:, PB * NCHUNK : 2 * PB * NCHUNK].rearrange(
                "p (b n) -> p b n", b=PB
            ),
            sqt[:],
            axis=mybir.AxisListType.X,
        )

        # ---- scales on ACT (single table set):
        #   scl[:,0:8]  = exp(-0.5*ln(0.25*nv2)) = 2/||v||   (temp folded)
        #   scl[:,8:16] = exp(-0.5*ln(nt2))      = 1/||t||
        lnall = stats.tile([P, 2 * PB * NCHUNK], F32, tag="lnall")
        scl = stats.tile([P, 2 * PB * NCHUNK], F32, tag="scl")
        nc.scalar.activation(
            lnall[:, 0 : PB * NCHUNK],
            nvt2[:, 0 : PB * NCHUNK],
            mybir.ActivationFunctionType.Ln,
            scale=0.25,
        )
        nc.scalar.activation(
            lnall[:, PB * NCHUNK :],
            nvt2[:, PB * NCHUNK :],
            mybir.ActivationFunctionType.Ln,
        )
        nc.scalar.activation(
            scl[:], lnall[:], mybir.ActivationFunctionType.Exp, scale=-0.5
        )

        # ---- T_hat = T * tsc  (one op; tsc broadcast along d via stride-0) --
        Th = normed.tile([P, PB, NCHUNK, D], BF16, tag="Th")
        tsc_bcast = _bcast_cols(scl, PB * NCHUNK, PB, NCHUNK, D)
        nc.vector.tensor_tensor(
            out=Th[:], in0=T[:], in1=tsc_bcast, op=mybir.AluOpType.mult
        )

        # ---- positive logits: plog = sv * sum_d V*Th ----
        sqp = scratch.tile([P, PB, NCHUNK, D], BF16, tag="sq")
        nc.vector.tensor_mul(sqp[:], V[:], Th[:])
        posd = stats.tile([P, PB * NCHUNK], BF16, tag="posd")
        nc.vector.reduce_sum(
            posd.rearrange("p (b n) -> p b n", b=PB),
            sqp[:],
            axis=mybir.AxisListType.X,
        )
        nc.vector.tensor_mul(
            plog_all[:, b0 * NCHUNK : (b0 + PB) * NCHUNK],
            posd[:],
            scl[:, 0 : PB * NCHUNK],
        )

        # ---- transpose T_hat ----
        Tt = trans.tile([P, PB, NDHALF, C], BF16, tag="Tt")
        for pb in range(PB):
            tpt = tp_pool.tile([P, NDHALF, C], BF16, tag="tp")
            for e in range(NDHALF):
                for j in range(NCHUNK):
                    nc.tensor.transpose(
                        tpt[:, e, j * P : (j + 1) * P],
                        Th[:, pb, j, e * P : (e + 1) * P],
                        identity,
                    )
            nc.any.tensor_copy(out=Tt[:, pb], in_=tpt[:])

        # ---- Gram chunks + fused exp/row-sum ----
        for pb in range(PB):
            b = b0 + pb
            for j in range(NCHUNK):
                gp = gp_pool.tile([P, C], F32, tag="gp")
                nc.tensor.matmul(
                    gp[:],
                    lhsT=Vt[:, pb, 0, j * P : (j + 1) * P],
                    rhs=Tt[:, pb, 0, :],
                    start=True,
                    stop=False,
                )
                nc.tensor.matmul(
                    gp[:],
                    lhsT=Vt[:, pb, 1, j * P : (j + 1) * P],
                    rhs=Tt[:, pb, 1, :],
                    start=False,
                    stop=True,
                )
                E = scratch.tile([P, C], BF16, tag="E")
                nc.scalar.activation(
                    E[:],
                    gp[:],
                    mybir.ActivationFunctionType.Exp,
                    scale=scl[:, pb * NCHUNK + j : pb * NCHUNK + j + 1],
                    accum_out=rs_all[:, b * NCHUNK + j : b * NCHUNK + j + 1],
                )

    # ---- hoisted finals: loss = ln(rowsum) - plog, one op each ----
    lnr = singles.tile([P, NCHUNK * B_PER_CORE], F32)
    nc.scalar.activation(lnr[:], rs_all[:], mybir.ActivationFunctionType.Ln)
    nc.vector.tensor_sub(loss_cols[:], lnr[:], plog_all[:])
    nc.sync.dma_start(out=loss_ap, in_=loss_cols[:])


_NC_CACHE = []


def _get_nc():
    if not _NC_CACHE:
        nc = bacc.Bacc("TRN2", target_bir_lowering=False, debug=False)
        v_dram = nc.dram_tensor("v", [B_PER_CORE, C, D], F32, kind="ExternalInput")
        t_dram = nc.dram_tensor("t", [B_PER_CORE, C, D], F32, kind="ExternalInput")
        loss_dram = nc.dram_tensor(
            "loss", [P, NCHUNK * B_PER_CORE], F32, kind="ExternalOutput"
        )
        with tile.TileContext(nc) as tc, ExitStack() as ctx:
            _emit(ctx, tc, loss_dram.ap(), v_dram.ap(), t_dram.ap())
        nc.compile()
        _NC_CACHE.append(nc)
    return _NC_CACHE[0]


def kernel(visual_features, text_embeddings):
    v = np.ascontiguousarray(np.asarray(visual_features, dtype=np.float32))
    t = np.ascontiguousarray(np.asarray(text_embeddings, dtype=np.float32))
    v = v.reshape(N_CORES, B_PER_CORE, C, D)
    t = t.reshape(N_CORES, B_PER_CORE, C, D)
    in_maps = [{"v": v[i], "t": t[i]} for i in range(N_CORES)]
    nc = _get_nc()
    res = run_bass_kernel_spmd(nc, in_maps, list(range(N_CORES)))
    total = 0.0
    for r in res.results:
        total += float(r["loss"].astype(np.float64).sum())
    return np.float32(total / (N_CORES * B_PER_CORE * C))

